# revision 12
# baseline (speedup 1.0000x reference)
"""Trainium2 Bass kernel for the CA2 dense-transformer problem.

Math (per batch b of 8, S=2048, D=512):
    Q1 = X @ W_xq.T + b_xq            # [S, D]
    Q2 = Y @ W_yq.T + b_yq
    Qc = concat(Q1, Q2, -1)           # [S, 2D]
    K  = Qc @ W_fk.T + b_fk
    V  = Qc @ W_fv.T + b_fv
    out = X + Y + softmax(Q1 K^T / sqrt(D)) V + softmax(Q2 K^T / sqrt(D)) V

Sharding: pure data-parallel over batch; core i handles batch i.

Numerics: every matmul runs in fp8e4 (e4m3) with DoubleRow perf mode,
accumulating in fp32 PSUM.  Weights are pre-scaled by 2^12 on the host so
their small uniform entries land in e4m3's normal range; the 2^-12
descale is folded into the fp32 epilogue.  The attention 1/sqrt(D) scale
is folded into the Exp activation's scale operand.  The softmax
denominator strip is bf16; the residual X+Y is bf16 on the host (the
residual dominates the output, bf16 rounding adds ~1e-3 rel err, well
under the 2e-2 gate); accumulation and output stay fp32.

Schedule: the Activation engine is the kernel bottleneck (softmax exp
over 2x2048x2048 scores), so exp instructions are fused pairwise where
PSUM banks allow: scores stream through 3 PSUM banks as a repeating
[single, fused-pair] pattern (S,D,D per 3 key-tiles), giving 5 fused +
6 single exps per 512-query block instead of 16 singles.  Projection
epilogues split evenly between Act and DVE; V epilogue and the po
normalizes run on DVE; residual init and racc accumulation on GPSIMD
(Pool).  All host-side tensors are partition-major so every DMA is a
contiguous per-partition copy; x/y residuals load last (first needed
mid-kernel) and per-512-token xt/yt slices interleave so the projection
pipeline never waits on DMA.
"""

import sys

if "/opt/trn_rl_repo" not in sys.path:
    sys.path.insert(0, "/opt/trn_rl_repo")

import ml_dtypes
import numpy as np

import concourse.bass as bass  # noqa: F401  (bass types used via tile/bacc)
import concourse.mybir as mybir
import concourse.tile as tile
from concourse import bacc
from concourse.bass_utils import run_bass_kernel_spmd

P = 128          # SBUF partitions
S = 2048         # tokens per batch
D = 512          # feature dim
NQT = S // P     # 16 token tiles
NET = D // P     # 4 feature tiles of D
NCT = 2 * D // P # 8 feature tiles of 2D
NE2 = NET // 2   # 2 double (256-deep) feature tiles of D
NC2 = NCT // 2   # 4 double feature tiles of 2D
NK2 = NQT // 2   # 8 double key tiles
NSS = S // 512   # 4 512-wide token column slices
QB = 512         # q-block columns processed together in attention
NQB = S // QB    # 4
NQS = QB // P    # 4 q-subtiles per block
FP = mybir.dt.float32
BF = mybir.dt.bfloat16
F8 = mybir.dt.float8e4
DR = mybir.MatmulPerfMode.DoubleRow
WS = 2.0 ** 12   # host-side weight pre-scale (max |w|*WS ~ 181 < 240)
IWS = 1.0 / WS

_CACHE = {}


def _build(reps: int = 1):
    nc = bacc.Bacc("TRN2", target_bir_lowering=False, debug=False)

    # All DRAM layouts are partition-major ([P, ...]) so DMAs are plain
    # contiguous per-partition copies (minimal descriptor work).
    xt_d = nc.dram_tensor("xt", [P, NET, S], F8, kind="ExternalInput")
    yt_d = nc.dram_tensor("yt", [P, NET, S], F8, kind="ExternalInput")
    x_d = nc.dram_tensor("x", [P, NQT, D], BF, kind="ExternalInput")
    y_d = nc.dram_tensor("y", [P, NQT, D], BF, kind="ExternalInput")
    wxq_d = nc.dram_tensor("wxq", [P, NET, D], F8, kind="ExternalInput")
    wyq_d = nc.dram_tensor("wyq", [P, NET, D], F8, kind="ExternalInput")
    wfk_d = nc.dram_tensor("wfk", [P, NCT, D], F8, kind="ExternalInput")
    wfv_d = nc.dram_tensor("wfv", [P, NCT, D], F8, kind="ExternalInput")
    bq_d = nc.dram_tensor("bq", [P, 12], FP, kind="ExternalInput")
    bfv_d = nc.dram_tensor("bfv", [P, D], FP, kind="ExternalInput")
    out_d = nc.dram_tensor("out", [NQT, P, D], FP, kind="ExternalOutput")

    Exp = mybir.ActivationFunctionType.Exp
    Ident = mybir.ActivationFunctionType.Identity
    mult = mybir.AluOpType.mult
    add = mybir.AluOpType.add
    ATT_SCALE = float(1.0 / np.sqrt(np.float32(D)))

    with tile.TileContext(nc) as tc:
        for _rep in range(reps):
            with (
                tc.tile_pool(name="main", bufs=1) as main,
            ):
                q1t = main.tile([P, NET, S], F8, tag="q1t")
                q2t = main.tile([P, NET, S], F8, tag="q2t")
                kft = main.tile([P, NET, S], F8, tag="kft")
                vf = main.tile([P, NQT, D], F8, tag="vf")
                racc = main.tile([P, NQT, D], FP, tag="racc")
                xres = main.tile([P, NQT, D], BF, tag="xres")
                yres = main.tile([P, NQT, D], BF, tag="yres")
                bq = main.tile([P, 12], FP, tag="bq")
                bfv = main.tile([P, D], FP, tag="bfv")
                # DoubleRow ldweights requires the k-pair dim stride to be a
                # multiple of 16 elements, so pad the ones tile to [P, 2, 16].
                ones8 = main.tile([P, 2, 16], F8, tag="ones8")
                nc.scalar.dma_start(bq[:], bq_d[:])
                nc.scalar.dma_start(bfv[:], bfv_d[:])
                nc.vector.memset(ones8[:], 1.0)

                with tc.tile_pool(name="stA", bufs=1) as stA:
                    xt = stA.tile([P, NET, S], F8, tag="xt")
                    yt = stA.tile([P, NET, S], F8, tag="yt")
                    wxq = stA.tile([P, NET, D], F8, tag="wxq")
                    wyq = stA.tile([P, NET, D], F8, tag="wyq")
                    wfk = stA.tile([P, NCT, D], F8, tag="wfk")
                    wfv = stA.tile([P, NCT, D], F8, tag="wfv")
                    # SP HWDGE queue: per-512-token xt/yt slices interleaved
                    # in first-use order; x/y residuals (first needed
                    # mid-kernel) last.  Scalar HWDGE queue (parallel): K/V
                    # weights + biases.
                    nc.sync.dma_start(wxq[:], wxq_d[:])
                    nc.sync.dma_start(xt[:, :, 0:512], xt_d[:, :, 0:512])
                    nc.sync.dma_start(wyq[:], wyq_d[:])
                    nc.sync.dma_start(yt[:, :, 0:512], yt_d[:, :, 0:512])
                    nc.scalar.dma_start(wfk[:], wfk_d[:])
                    nc.scalar.dma_start(wfv[:], wfv_d[:])
                    for ss in range(1, NSS):
                        sl = slice(ss * 512, (ss + 1) * 512)
                        nc.sync.dma_start(xt[:, :, sl], xt_d[:, :, sl])
                        nc.sync.dma_start(yt[:, :, sl], yt_d[:, :, sl])
                    nc.sync.dma_start(xres[:], x_d[:])
                    nc.sync.dma_start(yres[:], y_d[:])
                    # Residual init on GPSIMD (bf16 + bf16 -> fp32).
                    for kt in range(NQT):
                        nc.gpsimd.tensor_add(
                            racc[:, kt], xres[:, kt], yres[:, kt]
                        )

                    # Projections, pipelined per 512-token slice: Qx, Qy for
                    # slice ss, then K^T and V for the same tokens.  Epilogue
                    # = psum*1/WS + bias, cast to fp8; split evenly between
                    # Act (per-partition bias activation) and DVE.
                    with tc.tile_pool(name="psP", bufs=6, space="PSUM") as psP:
                        for ss in range(NSS):
                            sl = slice(ss * 512, (ss + 1) * 512)
                            for si, (src, w, qdst, bcol) in enumerate((
                                (xt, wxq, q1t, 0),
                                (yt, wyq, q2t, 4),
                            )):
                                for et in range(NET):
                                    ps = psP.tile(
                                        [P, 512], FP, tag="psP", name="psP"
                                    )
                                    for d2 in range(NE2):
                                        nc.tensor.matmul(
                                            ps[:],
                                            (w[:, 2 * d2 : 2 * d2 + 2, et * P : (et + 1) * P]),
                                            (src[:, 2 * d2 : 2 * d2 + 2, sl]),
                                            start=d2 == 0,
                                            stop=d2 == NE2 - 1,
                                            perf_mode=DR,
                                        )
                                    if (si + et) % 2 == 0:
                                        nc.scalar.activation(
                                            qdst[:, et, sl], ps[:], Ident,
                                            bias=bq[:, bcol + et : bcol + et + 1],
                                            scale=IWS,
                                        )
                                    else:
                                        nc.vector.tensor_scalar(
                                            qdst[:, et, sl], ps[:], IWS,
                                            bq[:, bcol + et : bcol + et + 1],
                                            mult, add,
                                        )
                            for et in range(NET):
                                ps = psP.tile([P, 512], FP, tag="psP", name="psP")
                                for c2 in range(NC2):
                                    qc = q1t if c2 < NE2 else q2t
                                    co = (2 * c2) % NET
                                    nc.tensor.matmul(
                                        ps[:],
                                        (wfk[:, 2 * c2 : 2 * c2 + 2, et * P : (et + 1) * P]),
                                        (qc[:, co : co + 2, sl]),
                                        start=c2 == 0,
                                        stop=c2 == NC2 - 1,
                                        perf_mode=DR,
                                    )
                                nc.scalar.activation(
                                    kft[:, et, sl], ps[:], Ident,
                                    bias=bq[:, 8 + et : 9 + et],
                                    scale=IWS,
                                )
                            for kt in range(4 * ss, 4 * ss + 4):
                                ps = psP.tile([P, D], FP, tag="psP", name="psP")
                                for c2 in range(NC2):
                                    qc = q1t if c2 < NE2 else q2t
                                    co = (2 * c2) % NET
                                    nc.tensor.matmul(
                                        ps[:],
                                        (qc[:, co : co + 2, kt * P : (kt + 1) * P]),
                                        (wfv[:, 2 * c2 : 2 * c2 + 2]),
                                        start=c2 == 0,
                                        stop=c2 == NC2 - 1,
                                        perf_mode=DR,
                                    )
                                nc.vector.scalar_tensor_tensor(
                                    vf[:, kt], ps[:], IWS, bfv[:],
                                    op0=mult, op1=add,
                                )

                # ---- Attention passes (shared K/V, fp8 DoubleRow) ----
                # PSUM: 4 po accumulators + 2 double-bank score tags = 8.
                # The Tile scheduler is dependency-driven (emission order is
                # irrelevant), and a score bank's WAR releases once the exp
                # has READ it (~240 ns in), so two alternating [P, 2, QB]
                # score tiles fully pipeline PE against Act while every exp
                # is a fused [128, 2, 512] instruction -- the Act engine
                # (kernel bottleneck) runs 8 fused exps per 512-query block.
                # Denominators are es-stationary ones-column matmuls ([P,1]
                # outputs, ~free on PE) accumulated in a recycled score-tag
                # slot at block end; reciprocal on DVE; normalize+residual
                # accumulate fused into one DVE scalar_tensor_tensor per
                # q-subtile (racc = po*rec + racc).
                with (
                    tc.tile_pool(name="esp", bufs=2) as esp,
                    tc.tile_pool(name="rcp", bufs=2) as rcp,
                    tc.tile_pool(name="psD1", bufs=1, space="PSUM") as psD1,
                    tc.tile_pool(name="psD2", bufs=1, space="PSUM") as psD2,
                    tc.tile_pool(name="pso", bufs=1, space="PSUM") as pso,
                ):
                    for qi, qsrc in enumerate((q1t, q2t)):
                        for qb in range(NQB):
                            qsl = slice(qb * QB, (qb + 1) * QB)
                            es = esp.tile([P, NQT, QB], F8, tag="es", name="es")

                            def scores_mm(kt, dst):
                                for e2 in range(NE2):
                                    nc.tensor.matmul(
                                        dst,
                                        (kft[:, 2 * e2 : 2 * e2 + 2, kt * P : (kt + 1) * P]),
                                        (qsrc[:, 2 * e2 : 2 * e2 + 2, qsl]),
                                        start=e2 == 0,
                                        stop=e2 == NE2 - 1,
                                        perf_mode=DR,
                                    )

                            for g in range(NK2):
                                pool = psD1 if g % 2 == 0 else psD2
                                d = pool.tile(
                                    [P, 2, QB], FP, tag="d", name="d"
                                )
                                scores_mm(2 * g, d[:, 0])
                                scores_mm(2 * g + 1, d[:, 1])
                                nc.scalar.activation(
                                    es[:, 2 * g : 2 * g + 2], d[:], Exp,
                                    scale=ATT_SCALE,
                                )

                            # Denominator burst in a recycled D2 slot.
                            dn = psD2.tile([P, 2, QB], FP, tag="d", name="dn")
                            for qs in range(NQS):
                                for k2 in range(NK2):
                                    nc.tensor.matmul(
                                        dn[:, 0, qs : qs + 1],
                                        (es[:, 2 * k2 : 2 * k2 + 2, qs * P : (qs + 1) * P]),
                                        (ones8[:, :, 0:1]),
                                        start=k2 == 0,
                                        stop=k2 == NK2 - 1,
                                        perf_mode=DR,
                                    )
                            rec = rcp.tile([P, NQS], FP, tag="rec", name="rec")
                            nc.vector.reciprocal(rec[:], dn[:, 0, 0:NQS])

                            for qs in range(NQS):
                                po = pso.tile(
                                    [P, D], FP, name=f"po{qs}", tag=f"po{qs}"
                                )
                                for k2 in range(NK2):
                                    nc.tensor.matmul(
                                        po[:],
                                        (es[:, 2 * k2 : 2 * k2 + 2, qs * P : (qs + 1) * P]),
                                        (vf[:, 2 * k2 : 2 * k2 + 2]),
                                        start=k2 == 0,
                                        stop=k2 == NK2 - 1,
                                        perf_mode=DR,
                                    )
                                qt_i = qb * NQS + qs
                                nc.vector.scalar_tensor_tensor(
                                    racc[:, qt_i],
                                    po[:],
                                    rec[:, qs : qs + 1],
                                    racc[:, qt_i],
                                    op0=mult,
                                    op1=add,
                                )
                                if qi == 1:
                                    nc.sync.dma_start(
                                        out_d[qt_i], racc[:, qt_i]
                                    )

    nc.compile()
    return nc


def get_nc(reps: int = 1):
    if reps not in _CACHE:
        _CACHE[reps] = _build(reps)
    return _CACHE[reps]


def make_in_maps(X, Y, W_xq, b_xq, W_yq, b_yq, W_fk, b_fk, W_fv, b_fv):
    """Host-side layout prep (transposes / fp8 quantization; weights
    pre-scaled by WS; everything partition-major) and per-core sharding
    over batch."""
    f32 = np.float32

    def q8(a):
        return np.ascontiguousarray(
            np.asarray(a, dtype=f32), dtype=ml_dtypes.float8_e4m3
        )

    def pmaj(a, n, w):
        # [n, P, w] -> [P, n, w] contiguous
        return np.ascontiguousarray(a.reshape(n, P, w).transpose(1, 0, 2))

    wxq = pmaj(q8(W_xq.T * WS), NET, D)
    wyq = pmaj(q8(W_yq.T * WS), NET, D)
    wfk = pmaj(q8(W_fk.T * WS), NCT, D)
    wfv = pmaj(q8(W_fv.T * WS), NCT, D)
    bq = np.empty((P, 12), f32)
    bq[:, 0:4] = b_xq.reshape(NET, P).T
    bq[:, 4:8] = b_yq.reshape(NET, P).T
    bq[:, 8:12] = b_fk.reshape(NET, P).T
    bfv = np.ascontiguousarray(
        np.broadcast_to(np.asarray(b_fv, f32), (P, D))
    )
    in_maps = []
    for b in range(X.shape[0]):
        xb = np.asarray(X[b], f32)
        yb = np.asarray(Y[b], f32)
        in_maps.append(
            {
                "xt": pmaj(q8(xb.T), NET, S),
                "yt": pmaj(q8(yb.T), NET, S),
                "x": np.ascontiguousarray(
                    xb.reshape(NQT, P, D).transpose(1, 0, 2),
                    dtype=ml_dtypes.bfloat16,
                ),
                "y": np.ascontiguousarray(
                    yb.reshape(NQT, P, D).transpose(1, 0, 2),
                    dtype=ml_dtypes.bfloat16,
                ),
                "wxq": wxq,
                "wyq": wyq,
                "wfk": wfk,
                "wfv": wfv,
                "bq": bq,
                "bfv": bfv,
            }
        )
    return in_maps


def kernel(X, Y, W_xq, b_xq, W_yq, b_yq, W_fk, b_fk, W_fv, b_fv):
    X = np.asarray(X, np.float32)
    Y = np.asarray(Y, np.float32)
    B = X.shape[0]
    nc = get_nc()
    in_maps = make_in_maps(
        X, Y,
        np.asarray(W_xq, np.float32), np.asarray(b_xq, np.float32),
        np.asarray(W_yq, np.float32), np.asarray(b_yq, np.float32),
        np.asarray(W_fk, np.float32), np.asarray(b_fk, np.float32),
        np.asarray(W_fv, np.float32), np.asarray(b_fv, np.float32),
    )
    res = run_bass_kernel_spmd(nc, in_maps, list(range(B)))
    out = np.stack([res.results[b]["out"].reshape(S, D) for b in range(B)])
    return out


# revision 13
# speedup vs baseline: 1.0720x; 1.0720x over previous
"""Trainium2 Bass kernel for the CA2 dense-transformer problem.

Math (per batch b of 8, S=2048, D=512):
    Q1 = X @ W_xq.T + b_xq            # [S, D]
    Q2 = Y @ W_yq.T + b_yq
    Qc = concat(Q1, Q2, -1)           # [S, 2D]
    K  = Qc @ W_fk.T + b_fk
    V  = Qc @ W_fv.T + b_fv
    out = X + Y + softmax(Q1 K^T / sqrt(D)) V + softmax(Q2 K^T / sqrt(D)) V

Sharding: pure data-parallel over batch; core i handles batch i.

Numerics: every matmul runs in fp8e4 (e4m3) with DoubleRow perf mode,
accumulating in fp32 PSUM.  Weights are pre-scaled by 2^12 on the host so
their small uniform entries land in e4m3's normal range; the 2^-12
descale is folded into the fp32 epilogue.  The attention 1/sqrt(D) scale
is folded into the Exp activation's scale operand.  The softmax
denominator strip is bf16; the residual X+Y is bf16 on the host (the
residual dominates the output, bf16 rounding adds ~1e-3 rel err, well
under the 2e-2 gate); accumulation and output stay fp32.

Schedule: the Activation engine is the kernel bottleneck (softmax exp
over 2x2048x2048 scores), so exp instructions are fused pairwise where
PSUM banks allow: scores stream through 3 PSUM banks as a repeating
[single, fused-pair] pattern (S,D,D per 3 key-tiles), giving 5 fused +
6 single exps per 512-query block instead of 16 singles.  Projection
epilogues split evenly between Act and DVE; V epilogue and the po
normalizes run on DVE; residual init and racc accumulation on GPSIMD
(Pool).  All host-side tensors are partition-major so every DMA is a
contiguous per-partition copy; x/y residuals load last (first needed
mid-kernel) and per-512-token xt/yt slices interleave so the projection
pipeline never waits on DMA.
"""

import sys

if "/opt/trn_rl_repo" not in sys.path:
    sys.path.insert(0, "/opt/trn_rl_repo")

import ml_dtypes
import numpy as np

import concourse.bass as bass  # noqa: F401  (bass types used via tile/bacc)
import concourse.mybir as mybir
import concourse.tile as tile
from concourse import bacc
from concourse.bass_utils import run_bass_kernel_spmd

P = 128          # SBUF partitions
S = 2048         # tokens per batch
D = 512          # feature dim
NQT = S // P     # 16 token tiles
NET = D // P     # 4 feature tiles of D
NCT = 2 * D // P # 8 feature tiles of 2D
NE2 = NET // 2   # 2 double (256-deep) feature tiles of D
NC2 = NCT // 2   # 4 double feature tiles of 2D
NK2 = NQT // 2   # 8 double key tiles
NSS = S // 512   # 4 512-wide token column slices
QB = 512         # q-block columns processed together in attention
NQB = S // QB    # 4
NQS = QB // P    # 4 q-subtiles per block
FP = mybir.dt.float32
BF = mybir.dt.bfloat16
F8 = mybir.dt.float8e4
DR = mybir.MatmulPerfMode.DoubleRow
WS = 2.0 ** 12   # host-side weight pre-scale (max |w|*WS ~ 181 < 240)
IWS = 1.0 / WS

_CACHE = {}


def _build(reps: int = 1):
    nc = bacc.Bacc("TRN2", target_bir_lowering=False, debug=False)

    # All DRAM layouts are partition-major ([P, ...]) so DMAs are plain
    # contiguous per-partition copies (minimal descriptor work).
    xt_d = nc.dram_tensor("xt", [P, NET, S], F8, kind="ExternalInput")
    yt_d = nc.dram_tensor("yt", [P, NET, S], F8, kind="ExternalInput")
    x_d = nc.dram_tensor("x", [P, NQT, D], BF, kind="ExternalInput")
    y_d = nc.dram_tensor("y", [P, NQT, D], BF, kind="ExternalInput")
    wxq_d = nc.dram_tensor("wxq", [P, NET, D], F8, kind="ExternalInput")
    wyq_d = nc.dram_tensor("wyq", [P, NET, D], F8, kind="ExternalInput")
    wfk_d = nc.dram_tensor("wfk", [P, NCT, D], F8, kind="ExternalInput")
    wfv_d = nc.dram_tensor("wfv", [P, NCT, D], F8, kind="ExternalInput")
    bq_d = nc.dram_tensor("bq", [P, 12], FP, kind="ExternalInput")
    bfv_d = nc.dram_tensor("bfv", [P, 2, D], FP, kind="ExternalInput")
    out_d = nc.dram_tensor("out", [NQT, P, D], FP, kind="ExternalOutput")

    Exp = mybir.ActivationFunctionType.Exp
    Ident = mybir.ActivationFunctionType.Identity
    mult = mybir.AluOpType.mult
    add = mybir.AluOpType.add
    ATT_SCALE = float(1.0 / np.sqrt(np.float32(D)))

    with tile.TileContext(nc) as tc:
        for _rep in range(reps):
            with (
                tc.tile_pool(name="main", bufs=1) as main,
            ):
                q1t = main.tile([P, NET, S], F8, tag="q1t")
                q2t = main.tile([P, NET, S], F8, tag="q2t")
                kft = main.tile([P, NET, S], F8, tag="kft")
                vf = main.tile([P, NQT, D], F8, tag="vf")
                racc = main.tile([P, NQT, D], FP, tag="racc")
                xres = main.tile([P, NQT, D], BF, tag="xres")
                yres = main.tile([P, NQT, D], BF, tag="yres")
                bq = main.tile([P, 12], FP, tag="bq")
                bfv = main.tile([P, 2, D], FP, tag="bfv")
                # DoubleRow ldweights requires the k-pair dim stride to be a
                # multiple of 16 elements, so pad the ones tile to [P, 2, 16].
                ones8 = main.tile([P, 2, 16], F8, tag="ones8")
                nc.scalar.dma_start(bq[:], bq_d[:])
                nc.scalar.dma_start(bfv[:], bfv_d[:])
                nc.vector.memset(ones8[:], 1.0)

                with tc.tile_pool(name="stA", bufs=1) as stA:
                    xt = stA.tile([P, NET, S], F8, tag="xt")
                    yt = stA.tile([P, NET, S], F8, tag="yt")
                    wxq = stA.tile([P, NET, D], F8, tag="wxq")
                    wyq = stA.tile([P, NET, D], F8, tag="wyq")
                    wfk = stA.tile([P, NCT, D], F8, tag="wfk")
                    wfv = stA.tile([P, NCT, D], F8, tag="wfv")
                    # SP HWDGE queue: per-512-token xt/yt slices interleaved
                    # in first-use order; x/y residuals (first needed
                    # mid-kernel) last.  Scalar HWDGE queue (parallel): K/V
                    # weights + biases.
                    nc.sync.dma_start(wxq[:], wxq_d[:])
                    nc.sync.dma_start(xt[:, :, 0:512], xt_d[:, :, 0:512])
                    nc.sync.dma_start(wyq[:], wyq_d[:])
                    nc.sync.dma_start(yt[:, :, 0:512], yt_d[:, :, 0:512])
                    nc.scalar.dma_start(wfk[:], wfk_d[:])
                    nc.scalar.dma_start(wfv[:], wfv_d[:])
                    for ss in range(1, NSS):
                        sl = slice(ss * 512, (ss + 1) * 512)
                        nc.sync.dma_start(xt[:, :, sl], xt_d[:, :, sl])
                        nc.sync.dma_start(yt[:, :, sl], yt_d[:, :, sl])
                    nc.sync.dma_start(xres[:], x_d[:])
                    nc.sync.dma_start(yres[:], y_d[:])
                    # Residual init on GPSIMD (bf16 + bf16 -> fp32).
                    for kt in range(NQT):
                        nc.gpsimd.tensor_add(
                            racc[:, kt], xres[:, kt], yres[:, kt]
                        )

                    # Projections, pipelined per 512-token slice: Qx,
                    # Qy, K^T, V.  PSUM flows through three [P, 2, 512]
                    # double-bank slots (tag psP) shared with the first two
                    # attention blocks' score tiles, so their fused exps
                    # stream during the projection tail and the Activation
                    # engine never drains.  Epilogue = psum*1/WS + bias, fp8
                    # out; K/Q epilogues mostly on DVE (Act is the global
                    # bottleneck), ss0's on Act (runs in Act's early idle
                    # window); V pairs as one scalar_tensor_tensor on DVE.
                    att = []

                    def scores_mm(qsrc, qsl, kt, dst):
                        for e2 in range(NE2):
                            nc.tensor.matmul(
                                dst,
                                (kft[:, 2 * e2 : 2 * e2 + 2, kt * P : (kt + 1) * P]),
                                (qsrc[:, 2 * e2 : 2 * e2 + 2, qsl]),
                                start=e2 == 0,
                                stop=e2 == NE2 - 1,
                                perf_mode=DR,
                            )

                    def emit_exps(pool, tagf, qi, qb, qsrc):
                        qsl = slice(qb * QB, (qb + 1) * QB)
                        es = esp.tile([P, NQT, QB], F8, tag="es", name="es")
                        for g in range(NK2):
                            d = pool(g).tile(
                                [P, 2, QB], FP, tag=tagf(g), name="d"
                            )
                            scores_mm(qsrc, qsl, 2 * g, d[:, 0])
                            scores_mm(qsrc, qsl, 2 * g + 1, d[:, 1])
                            nc.scalar.activation(
                                es[:, 2 * g : 2 * g + 2], d[:], Exp,
                                scale=ATT_SCALE,
                            )
                        return es

                    def emit_pv(denpool, qi, qb, es):
                        # Denominator burst in a recycled score slot, then
                        # reciprocal, per-qs PV accumulation and fused
                        # normalize-accumulate (racc = po*rec + racc).
                        dn = denpool.tile([P, 2, QB], FP, tag="d", name="dn")
                        for qs in range(NQS):
                            for k2 in range(NK2):
                                nc.tensor.matmul(
                                    dn[:, 0, qs : qs + 1],
                                    (es[:, 2 * k2 : 2 * k2 + 2, qs * P : (qs + 1) * P]),
                                    (ones8[:, :, 0:1]),
                                    start=k2 == 0,
                                    stop=k2 == NK2 - 1,
                                    perf_mode=DR,
                                )
                        rec = rcp.tile([P, NQS], FP, tag="rec", name="rec")
                        nc.vector.reciprocal(rec[:], dn[:, 0, 0:NQS])
                        for qs in range(NQS):
                            po = pso.tile(
                                [P, D], FP, name=f"po{qs}", tag=f"po{qs}"
                            )
                            for k2 in range(NK2):
                                nc.tensor.matmul(
                                    po[:],
                                    (es[:, 2 * k2 : 2 * k2 + 2, qs * P : (qs + 1) * P]),
                                    (vf[:, 2 * k2 : 2 * k2 + 2]),
                                    start=k2 == 0,
                                    stop=k2 == NK2 - 1,
                                    perf_mode=DR,
                                )
                            qt_i = qb * NQS + qs
                            nc.vector.scalar_tensor_tensor(
                                racc[:, qt_i],
                                po[:],
                                rec[:, qs : qs + 1],
                                racc[:, qt_i],
                                op0=mult,
                                op1=add,
                            )
                            if qi == 1:
                                nc.sync.dma_start(out_d[qt_i], racc[:, qt_i])

                    with (
                        tc.tile_pool(name="esp", bufs=3) as esp,
                        tc.tile_pool(name="rcp", bufs=2) as rcp,
                    ):
                        with tc.tile_pool(
                            name="psP", bufs=3, space="PSUM"
                        ) as psP:
                            for ss in range(NSS):
                                sl = slice(ss * 512, (ss + 1) * 512)
                                for si, (srct, w, qdst, bcol) in enumerate((
                                    (xt, wxq, q1t, 0),
                                    (yt, wyq, q2t, 4),
                                )):
                                    for e2 in range(NE2):
                                        ps = psP.tile(
                                            [P, 2, 512], FP, tag="psP",
                                            name="psP",
                                        )
                                        for h in range(2):
                                            et = 2 * e2 + h
                                            for d2 in range(NE2):
                                                nc.tensor.matmul(
                                                    ps[:, h],
                                                    (w[:, 2 * d2 : 2 * d2 + 2, et * P : (et + 1) * P]),
                                                    (srct[:, 2 * d2 : 2 * d2 + 2, sl]),
                                                    start=d2 == 0,
                                                    stop=d2 == NE2 - 1,
                                                    perf_mode=DR,
                                                )
                                            if ss == 0:
                                                nc.scalar.activation(
                                                    qdst[:, et, sl], ps[:, h],
                                                    Ident,
                                                    bias=bq[:, bcol + et : bcol + et + 1],
                                                    scale=IWS,
                                                )
                                            else:
                                                nc.vector.tensor_scalar(
                                                    qdst[:, et, sl], ps[:, h],
                                                    IWS,
                                                    bq[:, bcol + et : bcol + et + 1],
                                                    mult, add,
                                                )
                                for e2 in range(NE2):
                                    ps = psP.tile(
                                        [P, 2, 512], FP, tag="psP", name="psP"
                                    )
                                    for h in range(2):
                                        et = 2 * e2 + h
                                        for c2 in range(NC2):
                                            qc = q1t if c2 < NE2 else q2t
                                            co = (2 * c2) % NET
                                            nc.tensor.matmul(
                                                ps[:, h],
                                                (wfk[:, 2 * c2 : 2 * c2 + 2, et * P : (et + 1) * P]),
                                                (qc[:, co : co + 2, sl]),
                                                start=c2 == 0,
                                                stop=c2 == NC2 - 1,
                                                perf_mode=DR,
                                            )
                                        if ss == 0:
                                            nc.scalar.activation(
                                                kft[:, et, sl], ps[:, h],
                                                Ident,
                                                bias=bq[:, 8 + et : 9 + et],
                                                scale=IWS,
                                            )
                                        else:
                                            nc.vector.tensor_scalar(
                                                kft[:, et, sl], ps[:, h], IWS,
                                                bq[:, 8 + et : 9 + et],
                                                mult, add,
                                            )
                                for kp in range(2):
                                    kt = 4 * ss + 2 * kp
                                    ps = psP.tile(
                                        [P, 2, 512], FP, tag="psP", name="psP"
                                    )
                                    for h in range(2):
                                        for c2 in range(NC2):
                                            qc = q1t if c2 < NE2 else q2t
                                            co = (2 * c2) % NET
                                            nc.tensor.matmul(
                                                ps[:, h],
                                                (qc[:, co : co + 2, (kt + h) * P : (kt + h + 1) * P]),
                                                (wfv[:, 2 * c2 : 2 * c2 + 2]),
                                                start=c2 == 0,
                                                stop=c2 == NC2 - 1,
                                                perf_mode=DR,
                                            )
                                    nc.vector.scalar_tensor_tensor(
                                        vf[:, kt : kt + 2], ps[:], IWS,
                                        bfv[:],
                                        op0=mult, op1=add,
                                    )
                            # First two attention blocks' scores + exps ride
                            # the projection PSUM slots; their PV/den work
                            # runs after the dedicated pools open below.
                            for qi, qb in ((0, 0), (0, 1)):
                                att.append(
                                    (qi, qb, emit_exps(
                                        lambda g: psP, lambda g: "psP",
                                        qi, qb, q1t,
                                    ))
                                )

                        # ---- Attention (shared K/V, fp8 DoubleRow) ----
                        # PSUM: 4 po accumulators + 2 double-bank score tags.
                        # A score bank's WAR releases once its exp has read
                        # it, so two alternating [P, 2, QB] score tiles fully
                        # pipeline PE against Act; every exp is a fused
                        # [128, 2, 512] instruction.
                        with (
                            tc.tile_pool(name="psD1", bufs=1, space="PSUM") as psD1,
                            tc.tile_pool(name="psD2", bufs=1, space="PSUM") as psD2,
                            tc.tile_pool(name="pso", bufs=1, space="PSUM") as pso,
                        ):
                            for qi, qb, es in att:
                                emit_pv(psD2, qi, qb, es)
                            for qi, qsrc in enumerate((q1t, q2t)):
                                for qb in range(NQB):
                                    if qi == 0 and qb < 2:
                                        continue
                                    es = emit_exps(
                                        lambda g: psD1 if g % 2 == 0 else psD2,
                                        lambda g: "d",
                                        qi, qb, qsrc,
                                    )
                                    emit_pv(psD2, qi, qb, es)

    nc.compile()
    return nc


def get_nc(reps: int = 1):
    if reps not in _CACHE:
        _CACHE[reps] = _build(reps)
    return _CACHE[reps]


def make_in_maps(X, Y, W_xq, b_xq, W_yq, b_yq, W_fk, b_fk, W_fv, b_fv):
    """Host-side layout prep (transposes / fp8 quantization; weights
    pre-scaled by WS; everything partition-major) and per-core sharding
    over batch."""
    f32 = np.float32

    def q8(a):
        return np.ascontiguousarray(
            np.asarray(a, dtype=f32), dtype=ml_dtypes.float8_e4m3
        )

    def pmaj(a, n, w):
        # [n, P, w] -> [P, n, w] contiguous
        return np.ascontiguousarray(a.reshape(n, P, w).transpose(1, 0, 2))

    wxq = pmaj(q8(W_xq.T * WS), NET, D)
    wyq = pmaj(q8(W_yq.T * WS), NET, D)
    wfk = pmaj(q8(W_fk.T * WS), NCT, D)
    wfv = pmaj(q8(W_fv.T * WS), NCT, D)
    bq = np.empty((P, 12), f32)
    bq[:, 0:4] = b_xq.reshape(NET, P).T
    bq[:, 4:8] = b_yq.reshape(NET, P).T
    bq[:, 8:12] = b_fk.reshape(NET, P).T
    bfv = np.ascontiguousarray(
        np.broadcast_to(np.asarray(b_fv, f32), (P, D))
    )
    in_maps = []
    for b in range(X.shape[0]):
        xb = np.asarray(X[b], f32)
        yb = np.asarray(Y[b], f32)
        in_maps.append(
            {
                "xt": pmaj(q8(xb.T), NET, S),
                "yt": pmaj(q8(yb.T), NET, S),
                "x": np.ascontiguousarray(
                    xb.reshape(NQT, P, D).transpose(1, 0, 2),
                    dtype=ml_dtypes.bfloat16,
                ),
                "y": np.ascontiguousarray(
                    yb.reshape(NQT, P, D).transpose(1, 0, 2),
                    dtype=ml_dtypes.bfloat16,
                ),
                "wxq": wxq,
                "wyq": wyq,
                "wfk": wfk,
                "wfv": wfv,
                "bq": bq,
                "bfv": bfv,
            }
        )
    return in_maps


def kernel(X, Y, W_xq, b_xq, W_yq, b_yq, W_fk, b_fk, W_fv, b_fv):
    X = np.asarray(X, np.float32)
    Y = np.asarray(Y, np.float32)
    B = X.shape[0]
    nc = get_nc()
    in_maps = make_in_maps(
        X, Y,
        np.asarray(W_xq, np.float32), np.asarray(b_xq, np.float32),
        np.asarray(W_yq, np.float32), np.asarray(b_yq, np.float32),
        np.asarray(W_fk, np.float32), np.asarray(b_fk, np.float32),
        np.asarray(W_fv, np.float32), np.asarray(b_fv, np.float32),
    )
    res = run_bass_kernel_spmd(nc, in_maps, list(range(B)))
    out = np.stack([res.results[b]["out"].reshape(S, D) for b in range(B)])
    return out


# revision 26
# speedup vs baseline: 1.4082x; 1.3136x over previous
"""Trainium2 Bass kernel for the CA2 dense-transformer problem.

Math (per batch b of 8, S=2048, D=512):
    Q1 = X @ W_xq.T + b_xq            # [S, D]
    Q2 = Y @ W_yq.T + b_yq
    Qc = concat(Q1, Q2, -1)           # [S, 2D]
    K  = Qc @ W_fk.T + b_fk
    V  = Qc @ W_fv.T + b_fv
    out = X + Y + softmax(Q1 K^T / sqrt(D)) V + softmax(Q2 K^T / sqrt(D)) V

Sharding: pure data-parallel over batch; core i handles batch i.

Numerics: every matmul runs in fp8e4 (e4m3) with DoubleRow perf mode,
accumulating in fp32 PSUM.  Weights are pre-scaled by 2^12 on the host so
their small uniform entries land in e4m3's normal range; the 2^-12
descale is folded into the fp32 epilogue.  The attention 1/sqrt(D) scale
is folded into the Exp activation's scale operand.  The softmax
denominator strip is bf16; the residual X+Y is bf16 on the host (the
residual dominates the output, bf16 rounding adds ~1e-3 rel err, well
under the 2e-2 gate); accumulation and output stay fp32.

Schedule: the Activation engine is the kernel bottleneck (softmax exp
over 2x2048x2048 scores), so exp instructions are fused pairwise where
PSUM banks allow: scores stream through 3 PSUM banks as a repeating
[single, fused-pair] pattern (S,D,D per 3 key-tiles), giving 5 fused +
6 single exps per 512-query block instead of 16 singles.  Projection
epilogues split evenly between Act and DVE; V epilogue and the po
normalizes run on DVE; residual init and racc accumulation on GPSIMD
(Pool).  All host-side tensors are partition-major so every DMA is a
contiguous per-partition copy; x/y residuals load last (first needed
mid-kernel) and per-512-token xt/yt slices interleave so the projection
pipeline never waits on DMA.
"""

import sys
from contextlib import ExitStack

if "/opt/trn_rl_repo" not in sys.path:
    sys.path.insert(0, "/opt/trn_rl_repo")

import ml_dtypes
import numpy as np

import concourse.bass as bass  # noqa: F401  (bass types used via tile/bacc)
import concourse.mybir as mybir
import concourse.tile as tile
from concourse import bacc
from concourse.bass_utils import run_bass_kernel_spmd

P = 128          # SBUF partitions
S = 2048         # tokens per batch
D = 512          # feature dim
NQT = S // P     # 16 token tiles
NET = D // P     # 4 feature tiles of D
NCT = 2 * D // P # 8 feature tiles of 2D
NE2 = NET // 2   # 2 double (256-deep) feature tiles of D
NC2 = NCT // 2   # 4 double feature tiles of 2D
NK2 = NQT // 2   # 8 double key tiles
NSS = S // 512   # 4 512-wide token column slices
QB = 512         # q-block columns processed together in attention
NQB = S // QB    # 4
NQS = QB // P    # 4 q-subtiles per block
FP = mybir.dt.float32
BF = mybir.dt.bfloat16
F8 = mybir.dt.float8e4
DR = mybir.MatmulPerfMode.DoubleRow
WS = 2.0 ** 12   # host-side weight pre-scale (max |w|*WS ~ 181 < 240)
IWS = 1.0 / WS

_CACHE = {}


def _build(reps: int = 1):
    nc = bacc.Bacc("TRN2", target_bir_lowering=False, debug=False)

    # All DRAM layouts are partition-major ([P, ...]) so DMAs are plain
    # contiguous per-partition copies (minimal descriptor work).
    xt_d = nc.dram_tensor("xt", [P, NET, S], F8, kind="ExternalInput")
    yt_d = nc.dram_tensor("yt", [P, NET, S], F8, kind="ExternalInput")
    x_d = nc.dram_tensor("x", [P, NQT, D], BF, kind="ExternalInput")
    y_d = nc.dram_tensor("y", [P, NQT, D], BF, kind="ExternalInput")
    wxq_d = nc.dram_tensor("wxq", [P, NET, D], F8, kind="ExternalInput")
    wyq_d = nc.dram_tensor("wyq", [P, NET, D], F8, kind="ExternalInput")
    wfk_d = nc.dram_tensor("wfk", [P, NCT, D], F8, kind="ExternalInput")
    wfv_d = nc.dram_tensor("wfv", [P, NCT, D], F8, kind="ExternalInput")
    bq_d = nc.dram_tensor("bq", [P, 8], FP, kind="ExternalInput")
    out_d = nc.dram_tensor("out", [NQT, P, D], FP, kind="ExternalOutput")

    Exp = mybir.ActivationFunctionType.Exp
    Ident = mybir.ActivationFunctionType.Identity
    mult = mybir.AluOpType.mult
    add = mybir.AluOpType.add
    ATT_SCALE = float(1.0 / np.sqrt(np.float32(D)))

    with tile.TileContext(nc) as tc:
        for _rep in range(reps):
            with (
                tc.tile_pool(name="main", bufs=1) as main,
            ):
                q1t = main.tile([P, NET, S], F8, tag="q1t")
                q2t = main.tile([P, NET, S], F8, tag="q2t")
                kft = main.tile([P, NET, S], F8, tag="kft")
                vf = main.tile([P, NQT, D], F8, tag="vf")
                racc = main.tile([P, NQT, D], FP, tag="racc")
                xres = main.tile([P, NQT, D], BF, tag="xres")
                yres = main.tile([P, NQT, D], BF, tag="yres")
                bq = main.tile([P, 8], FP, tag="bq")
                # DoubleRow ldweights requires the k-pair dim stride to be a
                # multiple of 16 elements, so pad the ones tile to [P, 2, 16].
                ones8 = main.tile([P, 2, 16], F8, tag="ones8")
                nc.scalar.dma_start(bq[:], bq_d[:])
                nc.vector.memset(ones8[:], 1.0)

                with tc.tile_pool(name="stA", bufs=1) as stA:
                    xt = stA.tile([P, NET, S], F8, tag="xt")
                    yt = stA.tile([P, NET, S], F8, tag="yt")
                    wxq = stA.tile([P, NET, D], F8, tag="wxq")
                    wyq = stA.tile([P, NET, D], F8, tag="wyq")
                    wfk = stA.tile([P, NCT, D], F8, tag="wfk")
                    wfv = stA.tile([P, NCT, D], F8, tag="wfv")
                    # SP HWDGE queue: per-512-token xt/yt slices interleaved
                    # in first-use order; x/y residuals (first needed
                    # mid-kernel) last.  Scalar HWDGE queue (parallel): K/V
                    # weights + biases.
                    nc.sync.dma_start(wxq[:], wxq_d[:])
                    nc.sync.dma_start(xt[:, :, 0:512], xt_d[:, :, 0:512])
                    nc.sync.dma_start(wyq[:], wyq_d[:])
                    nc.sync.dma_start(yt[:, :, 0:512], yt_d[:, :, 0:512])
                    nc.scalar.dma_start(wfk[:], wfk_d[:])
                    nc.scalar.dma_start(wfv[:], wfv_d[:])
                    for ss in range(1, NSS):
                        sl = slice(ss * 512, (ss + 1) * 512)
                        nc.sync.dma_start(xt[:, :, sl], xt_d[:, :, sl])
                        nc.sync.dma_start(yt[:, :, sl], yt_d[:, :, sl])
                    nc.sync.dma_start(xres[:], x_d[:])
                    nc.sync.dma_start(yres[:], y_d[:])
                    # Residual init on GPSIMD (bf16 + bf16 -> fp32).
                    for kt in range(NQT):
                        nc.gpsimd.tensor_add(
                            racc[:, kt], xres[:, kt], yres[:, kt]
                        )

                    # Projections, pipelined per 512-token slice: Qx,
                    # Qy, K^T, V.  Epilogue = psum*1/WS (+ bias for Q only:
                    # the K bias cancels in softmax and the V bias is folded
                    # into the host-side residual), cast to fp8.  Q psums
                    # rotate through 4 single banks (per-partition bias =>
                    # unpairable); K and V psums use one double-bank tile
                    # each so their bias-free epilogues run as single
                    # [128, 2, 512] instructions.  Split: Q epis alternate
                    # Act/DVE, K pairs on Act, V pairs on DVE.
                    #
                    # Pool lifetimes are staged (ExitStack) so PSUM banks
                    # hand over without a phase barrier: psQ's 4 banks
                    # become the two attention score tags as soon as the Q
                    # projections drain, letting the first two attention
                    # blocks' fused exps overlap the K/V projection tail;
                    # psK/psV's 4 banks then become the po accumulators.
                    stA_ = ExitStack()
                    esp = stA_.enter_context(tc.tile_pool(name="esp", bufs=3))
                    rcp = stA_.enter_context(tc.tile_pool(name="rcp", bufs=2))
                    stP = ExitStack()
                    psQ = stP.enter_context(
                        tc.tile_pool(name="psQ", bufs=4, space="PSUM"))
                    psK = stP.enter_context(
                        tc.tile_pool(name="psK", bufs=1, space="PSUM"))
                    psV = stP.enter_context(
                        tc.tile_pool(name="psV", bufs=1, space="PSUM"))

                    for ss in range(NSS):
                        sl = slice(ss * 512, (ss + 1) * 512)
                        for si, (src_t, w, qdst, bcol) in enumerate((
                            (xt, wxq, q1t, 0),
                            (yt, wyq, q2t, 4),
                        )):
                            for et in range(NET):
                                ps = psQ.tile(
                                    [P, 512], FP, tag="psQ", name="psQ"
                                )
                                for d2 in range(NE2):
                                    nc.tensor.matmul(
                                        ps[:],
                                        (w[:, 2 * d2 : 2 * d2 + 2, et * P : (et + 1) * P]),
                                        (src_t[:, 2 * d2 : 2 * d2 + 2, sl]),
                                        start=d2 == 0,
                                        stop=d2 == NE2 - 1,
                                        perf_mode=DR,
                                    )
                                if (si + et) % 2 == 0:
                                    nc.scalar.activation(
                                        qdst[:, et, sl], ps[:], Ident,
                                        bias=bq[:, bcol + et : bcol + et + 1],
                                        scale=IWS,
                                    )
                                else:
                                    nc.vector.tensor_scalar(
                                        qdst[:, et, sl], ps[:], IWS,
                                        bq[:, bcol + et : bcol + et + 1],
                                        mult, add,
                                    )
                        for e2 in range(NE2):
                            ps = psK.tile(
                                [P, 2, 512], FP, tag="psK", name="psK"
                            )
                            for h in range(2):
                                et = 2 * e2 + h
                                for c2 in range(NC2):
                                    qc = q1t if c2 < NE2 else q2t
                                    co = (2 * c2) % NET
                                    nc.tensor.matmul(
                                        ps[:, h],
                                        (wfk[:, 2 * c2 : 2 * c2 + 2, et * P : (et + 1) * P]),
                                        (qc[:, co : co + 2, sl]),
                                        start=c2 == 0,
                                        stop=c2 == NC2 - 1,
                                        perf_mode=DR,
                                    )
                            nc.scalar.activation(
                                kft[:, 2 * e2 : 2 * e2 + 2, sl], ps[:],
                                Ident, scale=IWS,
                            )
                        for kp in range(2):
                            kt = 4 * ss + 2 * kp
                            ps = psV.tile(
                                [P, 2, 512], FP, tag="psV", name="psV"
                            )
                            for h in range(2):
                                for c2 in range(NC2):
                                    qc = q1t if c2 < NE2 else q2t
                                    co = (2 * c2) % NET
                                    nc.tensor.matmul(
                                        ps[:, h],
                                        (qc[:, co : co + 2, (kt + h) * P : (kt + h + 1) * P]),
                                        (wfv[:, 2 * c2 : 2 * c2 + 2]),
                                        start=c2 == 0,
                                        stop=c2 == NC2 - 1,
                                        perf_mode=DR,
                                    )
                            nc.vector.tensor_scalar_mul(
                                vf[:, kt : kt + 2], ps[:], IWS
                            )



                    # ---- Attention (shared K/V, fp8 DoubleRow) ----
                    # PSUM: 4 po accumulators + 2 double-bank score tags.
                    # A score bank's WAR releases once its exp has read it,
                    # so two alternating [P, 2, QB] score tiles fully
                    # pipeline PE against Act; every exp is a fused
                    # [128, 2, 512] instruction.  Denominators are
                    # es-stationary ones-column matmuls accumulated in a
                    # recycled score slot; normalize+accumulate fused into
                    # one DVE scalar_tensor_tensor per q-subtile.
                    psD = [None, None]

                    def emit_exps(qi, qb, qsrc):
                        qsl = slice(qb * QB, (qb + 1) * QB)
                        es = esp.tile([P, NQT, QB], F8, tag="es", name="es")
                        for g in range(NK2):
                            pool = psD[g % 2]
                            d = pool.tile([P, 2, QB], FP, tag="d", name="d")
                            for h in range(2):
                                kt = 2 * g + h
                                for e2 in range(NE2):
                                    nc.tensor.matmul(
                                        d[:, h],
                                        (kft[:, 2 * e2 : 2 * e2 + 2, kt * P : (kt + 1) * P]),
                                        (qsrc[:, 2 * e2 : 2 * e2 + 2, qsl]),
                                        start=e2 == 0,
                                        stop=e2 == NE2 - 1,
                                        perf_mode=DR,
                                    )
                            nc.scalar.activation(
                                es[:, 2 * g : 2 * g + 2], d[:], Exp,
                                scale=ATT_SCALE,
                            )
                        return es

                    def emit_pv(qi, qb, es):
                        dn = psD[1].tile([P, 2, QB], FP, tag="d", name="dn")
                        for qs in range(NQS):
                            for k2 in range(NK2):
                                nc.tensor.matmul(
                                    dn[:, 0, qs : qs + 1],
                                    (es[:, 2 * k2 : 2 * k2 + 2, qs * P : (qs + 1) * P]),
                                    (ones8[:, :, 0:1]),
                                    start=k2 == 0,
                                    stop=k2 == NK2 - 1,
                                    perf_mode=DR,
                                )
                        rec = rcp.tile([P, NQS], FP, tag="rec", name="rec")
                        nc.vector.reciprocal(rec[:], dn[:, 0, 0:NQS])
                        for qs in range(NQS):
                            po = pso.tile(
                                [P, D], FP, name=f"po{qs}", tag=f"po{qs}"
                            )
                            for k2 in range(NK2):
                                nc.tensor.matmul(
                                    po[:],
                                    (es[:, 2 * k2 : 2 * k2 + 2, qs * P : (qs + 1) * P]),
                                    (vf[:, 2 * k2 : 2 * k2 + 2]),
                                    start=k2 == 0,
                                    stop=k2 == NK2 - 1,
                                    perf_mode=DR,
                                )
                            qt_i = qb * NQS + qs
                            nc.vector.scalar_tensor_tensor(
                                racc[:, qt_i],
                                po[:],
                                rec[:, qs : qs + 1],
                                racc[:, qt_i],
                                op0=mult,
                                op1=add,
                            )
                            if qi == 1:
                                nc.sync.dma_start(out_d[qt_i], racc[:, qt_i])

                    stP.close()
                    stT = ExitStack()
                    psD[0] = stT.enter_context(
                        tc.tile_pool(name="psD1", bufs=1, space="PSUM"))
                    psD[1] = stT.enter_context(
                        tc.tile_pool(name="psD2", bufs=1, space="PSUM"))
                    pso = stT.enter_context(
                        tc.tile_pool(name="pso", bufs=1, space="PSUM"))

                    for qi, qsrc in enumerate((q1t, q2t)):
                        for qb in range(NQB):
                            es = emit_exps(qi, qb, qsrc)
                            emit_pv(qi, qb, es)

                    stT.close()
                    stA_.close()

    nc.compile()
    return nc


def get_nc(reps: int = 1):
    if reps not in _CACHE:
        _CACHE[reps] = _build(reps)
    return _CACHE[reps]


def make_in_maps(X, Y, W_xq, b_xq, W_yq, b_yq, W_fk, b_fk, W_fv, b_fv):
    """Host-side layout prep (transposes / fp8 quantization; weights
    pre-scaled by WS; everything partition-major) and per-core sharding
    over batch."""
    f32 = np.float32

    def q8(a):
        return np.ascontiguousarray(
            np.asarray(a, dtype=f32), dtype=ml_dtypes.float8_e4m3
        )

    def pmaj(a, n, w):
        # [n, P, w] -> [P, n, w] contiguous
        return np.ascontiguousarray(a.reshape(n, P, w).transpose(1, 0, 2))

    wxq = pmaj(q8(W_xq.T * WS), NET, D)
    wyq = pmaj(q8(W_yq.T * WS), NET, D)
    wfk = pmaj(q8(W_fk.T * WS), NCT, D)
    wfv = pmaj(q8(W_fv.T * WS), NCT, D)
    bq = np.empty((P, 8), f32)
    bq[:, 0:4] = b_xq.reshape(NET, P).T
    bq[:, 4:8] = b_yq.reshape(NET, P).T
    in_maps = []
    for b in range(X.shape[0]):
        xb = np.asarray(X[b], f32)
        yb = np.asarray(Y[b], f32)
        # V bias passes through softmax-weighted averaging as an exact
        # additive constant (attention rows sum to 1), once per pass.
        yb_res = yb + 2.0 * np.asarray(b_fv, f32)
        in_maps.append(
            {
                "xt": pmaj(q8(xb.T), NET, S),
                "yt": pmaj(q8(yb.T), NET, S),
                "x": np.ascontiguousarray(
                    xb.reshape(NQT, P, D).transpose(1, 0, 2),
                    dtype=ml_dtypes.bfloat16,
                ),
                "y": np.ascontiguousarray(
                    yb_res.reshape(NQT, P, D).transpose(1, 0, 2),
                    dtype=ml_dtypes.bfloat16,
                ),
                "wxq": wxq,
                "wyq": wyq,
                "wfk": wfk,
                "wfv": wfv,
                "bq": bq,
            }
        )
    return in_maps


def kernel(X, Y, W_xq, b_xq, W_yq, b_yq, W_fk, b_fk, W_fv, b_fv):
    X = np.asarray(X, np.float32)
    Y = np.asarray(Y, np.float32)
    B = X.shape[0]
    nc = get_nc()
    in_maps = make_in_maps(
        X, Y,
        np.asarray(W_xq, np.float32), np.asarray(b_xq, np.float32),
        np.asarray(W_yq, np.float32), np.asarray(b_yq, np.float32),
        np.asarray(W_fk, np.float32), np.asarray(b_fk, np.float32),
        np.asarray(W_fv, np.float32), np.asarray(b_fv, np.float32),
    )
    res = run_bass_kernel_spmd(nc, in_maps, list(range(B)))
    out = np.stack([res.results[b]["out"].reshape(S, D) for b in range(B)])
    return out


# revision 27
# speedup vs baseline: 7.7080x; 5.4738x over previous
"""Trainium2 Bass kernel for the CA2 dense-transformer problem.

Math (per batch b of 8, S=2048, D=512):
    Q1 = X @ W_xq.T + b_xq            # [S, D]
    Q2 = Y @ W_yq.T + b_yq
    Qc = concat(Q1, Q2, -1)           # [S, 2D]
    K  = Qc @ W_fk.T + b_fk
    V  = Qc @ W_fv.T + b_fv
    out = X + Y + softmax(Q1 K^T / sqrt(D)) V + softmax(Q2 K^T / sqrt(D)) V

Sharding: pure data-parallel over batch; core i handles batch i.

Algebraic reductions: the K bias adds a per-query constant to every
score, which softmax shift-invariance cancels exactly, so b_fk is
dropped.  The V bias passes through the attention average as an exact
additive constant (attention rows sum to 1), so 2*b_fv folds into the
host-side residual.  Both K and V epilogues thus become bias-free pure
scales, pairable into [128, 2, 512] instructions.

Numerics: every matmul runs in fp8e4 (e4m3) with DoubleRow perf mode,
accumulating in fp32 PSUM.  Weights are pre-scaled by 2^12 on the host
so their small uniform entries land in e4m3's normal range; the 2^-12
descale folds into the fp32 epilogues and the attention 1/sqrt(D) scale
into the Exp activation's scale operand.  The residual X+Y is bf16 on
the host (it dominates the output; bf16 rounding adds ~1e-3 rel err,
well under the 2e-2 gate); denominators, accumulation, and the output
stay fp32.

Schedule (the Tile scheduler is dependency-driven; only the DAG and the
pool/bank structure matter): the Activation engine is the kernel
bottleneck (softmax exp over 2x2048x2048 scores), so every exp is a
fused [128, 2, 512] instruction.  Attention PSUM: 4 po accumulator
banks + two alternating double-bank score tags; a score bank's WAR
releases once its exp has read it, which fully pipelines PE against
Act at 8 fused exps per 512-query block.  Denominators are es-stationary
ones-column matmuls ([128, 1] outputs, ~free on PE) accumulated in a
recycled score slot; reciprocal on DVE; normalize+residual-accumulate
fused into one DVE scalar_tensor_tensor per q-subtile.  Projections
pipeline per 512-token slice through 4 single-bank Q psums (per-
partition bias, Act/DVE alternating epilogues) plus one double-bank
tile each for K (paired pure-scale epilogue on Act) and V (paired on
DVE).  Residual init runs on GPSIMD from bf16 inputs.  All host-side
tensors are partition-major so every DMA is a contiguous per-partition
copy; x/y residuals load last (first needed mid-kernel) and per-512-
token xt/yt slices interleave so the projection pipeline rarely waits
on DMA.
"""

import sys
from contextlib import ExitStack

if "/opt/trn_rl_repo" not in sys.path:
    sys.path.insert(0, "/opt/trn_rl_repo")

import ml_dtypes
import numpy as np

import concourse.bass as bass  # noqa: F401  (bass types used via tile/bacc)
import concourse.mybir as mybir
import concourse.tile as tile
from concourse import bacc
from concourse.bass_utils import run_bass_kernel_spmd

P = 128          # SBUF partitions
S = 2048         # tokens per batch
D = 512          # feature dim
NQT = S // P     # 16 token tiles
NET = D // P     # 4 feature tiles of D
NCT = 2 * D // P # 8 feature tiles of 2D
NE2 = NET // 2   # 2 double (256-deep) feature tiles of D
NC2 = NCT // 2   # 4 double feature tiles of 2D
NK2 = NQT // 2   # 8 double key tiles
NSS = S // 512   # 4 512-wide token column slices
QB = 512         # q-block columns processed together in attention
NQB = S // QB    # 4
NQS = QB // P    # 4 q-subtiles per block
FP = mybir.dt.float32
BF = mybir.dt.bfloat16
F8 = mybir.dt.float8e4
DR = mybir.MatmulPerfMode.DoubleRow
WS = 2.0 ** 12   # host-side weight pre-scale (max |w|*WS ~ 181 < 240)
IWS = 1.0 / WS

_CACHE = {}


def _build(reps: int = 1):
    nc = bacc.Bacc("TRN2", target_bir_lowering=False, debug=False)

    # All DRAM layouts are partition-major ([P, ...]) so DMAs are plain
    # contiguous per-partition copies (minimal descriptor work).
    xt_d = nc.dram_tensor("xt", [P, NET, S], F8, kind="ExternalInput")
    yt_d = nc.dram_tensor("yt", [P, NET, S], F8, kind="ExternalInput")
    x_d = nc.dram_tensor("x", [P, NQT, D], BF, kind="ExternalInput")
    y_d = nc.dram_tensor("y", [P, NQT, D], BF, kind="ExternalInput")
    wxq_d = nc.dram_tensor("wxq", [P, NET, D], F8, kind="ExternalInput")
    wyq_d = nc.dram_tensor("wyq", [P, NET, D], F8, kind="ExternalInput")
    wfk_d = nc.dram_tensor("wfk", [P, NCT, D], F8, kind="ExternalInput")
    wfv_d = nc.dram_tensor("wfv", [P, NCT, D], F8, kind="ExternalInput")
    bq_d = nc.dram_tensor("bq", [P, 8], FP, kind="ExternalInput")
    out_d = nc.dram_tensor("out", [NQT, P, D], FP, kind="ExternalOutput")

    Exp = mybir.ActivationFunctionType.Exp
    Ident = mybir.ActivationFunctionType.Identity
    mult = mybir.AluOpType.mult
    add = mybir.AluOpType.add
    ATT_SCALE = float(1.0 / np.sqrt(np.float32(D)))

    with tile.TileContext(nc) as tc:
        for _rep in range(reps):
            with (
                tc.tile_pool(name="main", bufs=1) as main,
            ):
                q1t = main.tile([P, NET, S], F8, tag="q1t")
                q2t = main.tile([P, NET, S], F8, tag="q2t")
                kft = main.tile([P, NET, S], F8, tag="kft")
                vf = main.tile([P, NQT, D], F8, tag="vf")
                racc = main.tile([P, NQT, D], FP, tag="racc")
                xres = main.tile([P, NQT, D], BF, tag="xres")
                yres = main.tile([P, NQT, D], BF, tag="yres")
                bq = main.tile([P, 8], FP, tag="bq")
                # DoubleRow ldweights requires the k-pair dim stride to be a
                # multiple of 16 elements, so pad the ones tile to [P, 2, 16].
                ones8 = main.tile([P, 2, 16], F8, tag="ones8")
                nc.scalar.dma_start(bq[:], bq_d[:])
                nc.vector.memset(ones8[:], 1.0)

                with tc.tile_pool(name="stA", bufs=1) as stA:
                    xt = stA.tile([P, NET, S], F8, tag="xt")
                    yt = stA.tile([P, NET, S], F8, tag="yt")
                    wxq = stA.tile([P, NET, D], F8, tag="wxq")
                    wyq = stA.tile([P, NET, D], F8, tag="wyq")
                    wfk = stA.tile([P, NCT, D], F8, tag="wfk")
                    wfv = stA.tile([P, NCT, D], F8, tag="wfv")
                    # SP HWDGE queue: per-512-token xt/yt slices interleaved
                    # in first-use order; x/y residuals (first needed
                    # mid-kernel) last.  Scalar HWDGE queue (parallel): K/V
                    # weights + biases.
                    nc.sync.dma_start(wxq[:], wxq_d[:])
                    nc.sync.dma_start(xt[:, :, 0:512], xt_d[:, :, 0:512])
                    nc.sync.dma_start(wyq[:], wyq_d[:])
                    nc.sync.dma_start(yt[:, :, 0:512], yt_d[:, :, 0:512])
                    nc.scalar.dma_start(wfk[:], wfk_d[:])
                    nc.scalar.dma_start(wfv[:], wfv_d[:])
                    for ss in range(1, NSS):
                        sl = slice(ss * 512, (ss + 1) * 512)
                        nc.sync.dma_start(xt[:, :, sl], xt_d[:, :, sl])
                        nc.sync.dma_start(yt[:, :, sl], yt_d[:, :, sl])
                    nc.sync.dma_start(xres[:], x_d[:])
                    nc.sync.dma_start(yres[:], y_d[:])
                    # Residual init on GPSIMD (bf16 + bf16 -> fp32).
                    for kt in range(NQT):
                        nc.gpsimd.tensor_add(
                            racc[:, kt], xres[:, kt], yres[:, kt]
                        )

                    # Projections, pipelined per 512-token slice: Qx,
                    # Qy, K^T, V.  Epilogue = psum*1/WS (+ bias for Q only:
                    # the K bias cancels in softmax and the V bias is folded
                    # into the host-side residual), cast to fp8.  Q psums
                    # rotate through 4 single banks (per-partition bias =>
                    # unpairable); K and V psums use one double-bank tile
                    # each so their bias-free epilogues run as single
                    # [128, 2, 512] instructions.  Split: Q epis alternate
                    # Act/DVE, K pairs on Act, V pairs on DVE.
                    #
                    # Pool lifetimes are staged (ExitStack) so PSUM banks
                    # hand over without a phase barrier: psQ's 4 banks
                    # become the two attention score tags as soon as the Q
                    # projections drain, letting the first two attention
                    # blocks' fused exps overlap the K/V projection tail;
                    # psK/psV's 4 banks then become the po accumulators.
                    stA_ = ExitStack()
                    esp = stA_.enter_context(tc.tile_pool(name="esp", bufs=3))
                    rcp = stA_.enter_context(tc.tile_pool(name="rcp", bufs=2))
                    stP = ExitStack()
                    psQ = stP.enter_context(
                        tc.tile_pool(name="psQ", bufs=4, space="PSUM"))
                    psK = stP.enter_context(
                        tc.tile_pool(name="psK", bufs=1, space="PSUM"))
                    psV = stP.enter_context(
                        tc.tile_pool(name="psV", bufs=1, space="PSUM"))

                    for ss in range(NSS):
                        sl = slice(ss * 512, (ss + 1) * 512)
                        for si, (src_t, w, qdst, bcol) in enumerate((
                            (xt, wxq, q1t, 0),
                            (yt, wyq, q2t, 4),
                        )):
                            for et in range(NET):
                                ps = psQ.tile(
                                    [P, 512], FP, tag="psQ", name="psQ"
                                )
                                for d2 in range(NE2):
                                    nc.tensor.matmul(
                                        ps[:],
                                        (w[:, 2 * d2 : 2 * d2 + 2, et * P : (et + 1) * P]),
                                        (src_t[:, 2 * d2 : 2 * d2 + 2, sl]),
                                        start=d2 == 0,
                                        stop=d2 == NE2 - 1,
                                        perf_mode=DR,
                                    )
                                if (si + et) % 2 == 0:
                                    nc.scalar.activation(
                                        qdst[:, et, sl], ps[:], Ident,
                                        bias=bq[:, bcol + et : bcol + et + 1],
                                        scale=IWS,
                                    )
                                else:
                                    nc.vector.tensor_scalar(
                                        qdst[:, et, sl], ps[:], IWS,
                                        bq[:, bcol + et : bcol + et + 1],
                                        mult, add,
                                    )
                        for e2 in range(NE2):
                            ps = psK.tile(
                                [P, 2, 512], FP, tag="psK", name="psK"
                            )
                            for h in range(2):
                                et = 2 * e2 + h
                                for c2 in range(NC2):
                                    qc = q1t if c2 < NE2 else q2t
                                    co = (2 * c2) % NET
                                    nc.tensor.matmul(
                                        ps[:, h],
                                        (wfk[:, 2 * c2 : 2 * c2 + 2, et * P : (et + 1) * P]),
                                        (qc[:, co : co + 2, sl]),
                                        start=c2 == 0,
                                        stop=c2 == NC2 - 1,
                                        perf_mode=DR,
                                    )
                            nc.scalar.activation(
                                kft[:, 2 * e2 : 2 * e2 + 2, sl], ps[:],
                                Ident, scale=IWS,
                            )
                        for kp in range(2):
                            kt = 4 * ss + 2 * kp
                            ps = psV.tile(
                                [P, 2, 512], FP, tag="psV", name="psV"
                            )
                            for h in range(2):
                                for c2 in range(NC2):
                                    qc = q1t if c2 < NE2 else q2t
                                    co = (2 * c2) % NET
                                    nc.tensor.matmul(
                                        ps[:, h],
                                        (qc[:, co : co + 2, (kt + h) * P : (kt + h + 1) * P]),
                                        (wfv[:, 2 * c2 : 2 * c2 + 2]),
                                        start=c2 == 0,
                                        stop=c2 == NC2 - 1,
                                        perf_mode=DR,
                                    )
                            nc.vector.tensor_scalar_mul(
                                vf[:, kt : kt + 2], ps[:], IWS
                            )



                    # ---- Attention (shared K/V, fp8 DoubleRow) ----
                    # PSUM: 4 po accumulators + 2 double-bank score tags.
                    # A score bank's WAR releases once its exp has read it,
                    # so two alternating [P, 2, QB] score tiles fully
                    # pipeline PE against Act; every exp is a fused
                    # [128, 2, 512] instruction.  Denominators are
                    # es-stationary ones-column matmuls accumulated in a
                    # recycled score slot; normalize+accumulate fused into
                    # one DVE scalar_tensor_tensor per q-subtile.
                    psD = [None, None]

                    def emit_exps(qi, qb, qsrc):
                        qsl = slice(qb * QB, (qb + 1) * QB)
                        es = esp.tile([P, NQT, QB], F8, tag="es", name="es")
                        for g in range(NK2):
                            pool = psD[g % 2]
                            d = pool.tile([P, 2, QB], FP, tag="d", name="d")
                            for h in range(2):
                                kt = 2 * g + h
                                for e2 in range(NE2):
                                    nc.tensor.matmul(
                                        d[:, h],
                                        (kft[:, 2 * e2 : 2 * e2 + 2, kt * P : (kt + 1) * P]),
                                        (qsrc[:, 2 * e2 : 2 * e2 + 2, qsl]),
                                        start=e2 == 0,
                                        stop=e2 == NE2 - 1,
                                        perf_mode=DR,
                                    )
                            nc.scalar.activation(
                                es[:, 2 * g : 2 * g + 2], d[:], Exp,
                                scale=ATT_SCALE,
                            )
                        return es

                    def emit_pv(qi, qb, es):
                        dn = psD[1].tile([P, 2, QB], FP, tag="d", name="dn")
                        for qs in range(NQS):
                            for k2 in range(NK2):
                                nc.tensor.matmul(
                                    dn[:, 0, qs : qs + 1],
                                    (es[:, 2 * k2 : 2 * k2 + 2, qs * P : (qs + 1) * P]),
                                    (ones8[:, :, 0:1]),
                                    start=k2 == 0,
                                    stop=k2 == NK2 - 1,
                                    perf_mode=DR,
                                )
                        rec = rcp.tile([P, NQS], FP, tag="rec", name="rec")
                        nc.vector.reciprocal(rec[:], dn[:, 0, 0:NQS])
                        for qs in range(NQS):
                            po = pso.tile(
                                [P, D], FP, name=f"po{qs}", tag=f"po{qs}"
                            )
                            for k2 in range(NK2):
                                nc.tensor.matmul(
                                    po[:],
                                    (es[:, 2 * k2 : 2 * k2 + 2, qs * P : (qs + 1) * P]),
                                    (vf[:, 2 * k2 : 2 * k2 + 2]),
                                    start=k2 == 0,
                                    stop=k2 == NK2 - 1,
                                    perf_mode=DR,
                                )
                            qt_i = qb * NQS + qs
                            nc.vector.scalar_tensor_tensor(
                                racc[:, qt_i],
                                po[:],
                                rec[:, qs : qs + 1],
                                racc[:, qt_i],
                                op0=mult,
                                op1=add,
                            )
                            if qi == 1:
                                nc.sync.dma_start(out_d[qt_i], racc[:, qt_i])

                    stP.close()
                    stT = ExitStack()
                    psD[0] = stT.enter_context(
                        tc.tile_pool(name="psD1", bufs=1, space="PSUM"))
                    psD[1] = stT.enter_context(
                        tc.tile_pool(name="psD2", bufs=1, space="PSUM"))
                    pso = stT.enter_context(
                        tc.tile_pool(name="pso", bufs=1, space="PSUM"))

                    for qi, qsrc in enumerate((q1t, q2t)):
                        for qb in range(NQB):
                            es = emit_exps(qi, qb, qsrc)
                            emit_pv(qi, qb, es)

                    stT.close()
                    stA_.close()

    nc.compile()
    return nc


def get_nc(reps: int = 1):
    if reps not in _CACHE:
        _CACHE[reps] = _build(reps)
    return _CACHE[reps]


def make_in_maps(X, Y, W_xq, b_xq, W_yq, b_yq, W_fk, b_fk, W_fv, b_fv):
    """Host-side layout prep (transposes / fp8 quantization; weights
    pre-scaled by WS; everything partition-major) and per-core sharding
    over batch."""
    f32 = np.float32

    def q8(a):
        return np.ascontiguousarray(
            np.asarray(a, dtype=f32), dtype=ml_dtypes.float8_e4m3
        )

    def pmaj(a, n, w):
        # [n, P, w] -> [P, n, w] contiguous
        return np.ascontiguousarray(a.reshape(n, P, w).transpose(1, 0, 2))

    wxq = pmaj(q8(W_xq.T * WS), NET, D)
    wyq = pmaj(q8(W_yq.T * WS), NET, D)
    wfk = pmaj(q8(W_fk.T * WS), NCT, D)
    wfv = pmaj(q8(W_fv.T * WS), NCT, D)
    bq = np.empty((P, 8), f32)
    bq[:, 0:4] = b_xq.reshape(NET, P).T
    bq[:, 4:8] = b_yq.reshape(NET, P).T
    in_maps = []
    for b in range(X.shape[0]):
        xb = np.asarray(X[b], f32)
        yb = np.asarray(Y[b], f32)
        # V bias passes through softmax-weighted averaging as an exact
        # additive constant (attention rows sum to 1), once per pass.
        yb_res = yb + 2.0 * np.asarray(b_fv, f32)
        in_maps.append(
            {
                "xt": pmaj(q8(xb.T), NET, S),
                "yt": pmaj(q8(yb.T), NET, S),
                "x": np.ascontiguousarray(
                    xb.reshape(NQT, P, D).transpose(1, 0, 2),
                    dtype=ml_dtypes.bfloat16,
                ),
                "y": np.ascontiguousarray(
                    yb_res.reshape(NQT, P, D).transpose(1, 0, 2),
                    dtype=ml_dtypes.bfloat16,
                ),
                "wxq": wxq,
                "wyq": wyq,
                "wfk": wfk,
                "wfv": wfv,
                "bq": bq,
            }
        )
    return in_maps


def kernel(X, Y, W_xq, b_xq, W_yq, b_yq, W_fk, b_fk, W_fv, b_fv):
    X = np.asarray(X, np.float32)
    Y = np.asarray(Y, np.float32)
    B = X.shape[0]
    nc = get_nc()
    in_maps = make_in_maps(
        X, Y,
        np.asarray(W_xq, np.float32), np.asarray(b_xq, np.float32),
        np.asarray(W_yq, np.float32), np.asarray(b_yq, np.float32),
        np.asarray(W_fk, np.float32), np.asarray(b_fk, np.float32),
        np.asarray(W_fv, np.float32), np.asarray(b_fv, np.float32),
    )
    res = run_bass_kernel_spmd(nc, in_maps, list(range(B)))
    out = np.stack([res.results[b]["out"].reshape(S, D) for b in range(B)])
    return out
